# revision 48
# baseline (speedup 1.0000x reference)
"""Trainium2 Bass kernel for a Lorentz RGCN message-passing layer.

Strategy (8 NeuronCores, SPMD):
  - Nodes are range-partitioned: core c owns 6272 destination nodes
    (49 windows x 128).  All edges whose dst falls in a core's range are
    processed by that core, so no cross-core reduction is needed; each
    core writes a disjoint slice of the output.
  - Since NUM_BASES == D (SI=SO=1), the per-edge relation transform is
    elementwise: msg = h_tangent[src] * weight[etype] + rel_emb[etype].
  - fp16 value path for the edge phase (f32 accumulate); verified mean
    rel err ~2.6e-3 vs the f32 reference.
  - h rows are fetched via gpsimd.dma_gather directly into the msg tile:
    single 256B fp16 rows, int16 indices via a half-table split (rows
    < 25088 vs >= 25088, edges grouped per window by half on the host).
  - weight/rel_emb per edge are HOST-prebuilt into sequential fp16
    streams (pure input indexing), so no second gather is needed.
  - The exp0+to_lorentz scalar chain is reformulated as polynomials in
    n2=|msg|^2 (tanh(x)/x is even), removing all Sqrt/Tanh activation
    table loads from the edge phase.  Same for log0's artanh in phase A.
  - Segment sums: edges are bucketed on the host into their dst window
    (128 nodes); each 128-edge tile is reduced on the TensorEngine with a
    one-hot selection matrix into a PSUM accumulator [128 nodes x 130];
    the per-edge centroid weight sxi is folded into the rhs msg columns.
    PSUM column layout: [deg(=1), x0, sxi*msg].
  - The per-window tile split (kA half-A tiles, kB half-B tiles) is
    computed on the host from the actual inputs (max over the 8 cores,
    since the SPMD program is shared) and baked into the program; ops
    only touch used tiles.
  - Per-node epilogue (centroid normalization, log0, self-loop matmul,
    exp0) runs on 49 windows of 128 nodes with batched per-node scalars.
"""

import sys

sys.path.insert(0, "/opt/trn_rl_repo")

import numpy as np

import concourse.bass as bass
import concourse.bacc as bacc
import concourse.mybir as mybir
from concourse.tile import TileContext
from concourse.masks import make_identity

# ---------------------------------------------------------------- constants
NCORES = 8
N = 50000
E = 800000
D = 128
R = 230
C = 0.01
SC = 0.1  # sqrt(C)
EPS = 1e-7

NPC = 6272                 # nodes per core = 49 windows * 128
NW = 49                    # windows per core
NROT = NCORES * NPC        # 50176 rows in the (rolled, padded) h table
HALF = NROT // 2           # 25088

P1 = -C / 3.0              # tanh(sc n)/(sc n) ~= 1 + P1*n2
A1 = C / 3.0               # artanh(sc n)/(sc n) ~= 1 + A1*n2 + A2*n2^2
A2 = C * C / 5.0

f32 = mybir.dt.float32
f16 = mybir.dt.float16
i16 = mybir.dt.int16
i8 = mybir.dt.int8
i32 = mybir.dt.int32
OP = mybir.AluOpType
AF = mybir.ActivationFunctionType


# ------------------------------------------------------------ device program
_PROGRAM = None  # (nc, layout_key)


SW = 1  # windows per gather super-group


def _build_program(tpw, ka, kb):
    """ka/kb: per-window half-A/half-B tile counts (uniform across cores).

    Edge slots are packed tightly window-major: window w owns kU[w]=ka+kb
    tiles at offset woff[w] (A-half slots first, then B-half).  Gathers run
    per super-group of SW windows (all A-halves in one call, all B-halves
    in the next) to amortize the ~9.5us fixed DMAGatherAnt ucode cost.
    """
    ku = [a + b for a, b in zip(ka, kb)]
    woff = np.concatenate([[0], np.cumsum(ku)]).astype(int)  # per-window tile offset
    ntiles = int(woff[-1])
    idxc = ntiles * 128 // 16
    nc = bacc.Bacc("TRN2", target_bir_lowering=False, debug=False)

    h_roll = nc.declare_dram_parameter("h_roll", [128, NROT // 128, D], f16,
                                       isOutput=False)
    w_d = nc.declare_dram_parameter("w_e", [128, ntiles, D], f16, isOutput=False)
    r_d = nc.declare_dram_parameter("r_e", [128, ntiles, D], f16, isOutput=False)
    lw_d = nc.declare_dram_parameter("lw", [D, D], f32, isOutput=False)
    ev_d = nc.declare_dram_parameter("ev", [D, D], f32, isOutput=False)
    norm_d = nc.declare_dram_parameter("norm_c", [NPC, 1], f32, isOutput=False)
    idxh_d = nc.declare_dram_parameter("idx_h", [128, idxc], i16, isOutput=False)
    cnt_d = nc.declare_dram_parameter("cnt", [1, 2 * NW], i32, isOutput=False)
    drel_d = nc.declare_dram_parameter("drel", [128, ntiles], f32, isOutput=False)
    out_d = nc.declare_dram_parameter("out", [NPC, D], f32, isOutput=True)
    htab = nc.dram_tensor("htab", [NROT, D], f16)

    with TileContext(nc) as tc:
        with (
            tc.tile_pool(name="persist", bufs=1) as pp,
            tc.tile_pool(name="consts", bufs=1) as cp,
        ):
            S_all = pp.tile([128, NW, 130], f32)
            h_loc = pp.tile([128, NW, D], f32)
            hn = pp.tile([128, NW, D], f32)
            drel_sb = pp.tile([128, ntiles], f32)
            norm_sb = pp.tile([128, NW], f32)
            s2raw = pp.tile([128, NW], f32)
            ne2 = pp.tile([128, NW], f32)
            cnt_sb = pp.tile([1, 2 * NW], i32)
            nc.sync.dma_start(out=cnt_sb[:], in_=cnt_d[:])

            LW = cp.tile([128, D], f32)
            EV = cp.tile([128, D], f32)
            iota_f = cp.tile([128, 128], f32)
            ident = cp.tile([128, 128], f32)
            iota_i = cp.tile([128, 128], i32)

            nc.sync.dma_start(out=drel_sb[:], in_=drel_d[:])
            nc.sync.dma_start(
                out=norm_sb[:], in_=norm_d[:].rearrange("(w p) o -> p (w o)", p=128)
            )
            nc.sync.dma_start(out=LW[:], in_=lw_d[:])
            nc.sync.dma_start(out=EV[:], in_=ev_d[:])
            nc.gpsimd.iota(iota_i[:], pattern=[[1, 128]], base=0, channel_multiplier=0)
            nc.vector.tensor_copy(out=iota_f[:], in_=iota_i[:])
            make_identity(nc, ident[:])

            # ---------------- phase A: h_tangent table (log0 of h_roll) ----
            # scale = artanh(sc n)/(sc n) = 1 + A1*n2 + A2*n2^2 (poly, no tables)
            # h_roll is host-pre-tiled [128, 392, D]: cols [0:56) hold rows
            # T*128+p ((t,p)-tiled, feeds h_loc), cols [56:392) hold rows
            # 7168 + p*336 + j ((p,j)-tiled, big contiguous DMA bursts).
            SUP = 28
            NSUP = NROT // (SUP * 128)  # 14
            NP1 = 2                     # (t,p)-tiled supertiles (rows < 7168)
            htab_pj = htab[7168:NROT, :].rearrange("(p j) d -> p j d", p=128)
            with tc.tile_pool(name="phA", bufs=3) as pa:
                for s in range(NSUP):
                    xin = pa.tile([128, SUP, D], f16, tag="xin")
                    nc.sync.dma_start(
                        out=xin[:], in_=h_roll[:, s * SUP : (s + 1) * SUP, :]
                    )
                    sqa = pa.tile([128, SUP, D], f16, tag="sqa")
                    nc.vector.tensor_tensor(
                        out=sqa[:], in0=xin[:], in1=xin[:], op=OP.mult
                    )
                    n2a = pa.tile([128, SUP], f32, tag="n2a")
                    nc.vector.tensor_reduce(
                        out=n2a[:], in_=sqa[:], axis=mybir.AxisListType.X, op=OP.add
                    )
                    t1 = pa.tile([128, SUP], f32, tag="t1")
                    nc.vector.tensor_scalar(
                        out=t1[:], in0=n2a[:], scalar1=A2, scalar2=A1,
                        op0=OP.mult, op1=OP.add,
                    )
                    scl = pa.tile([128, SUP], f32, tag="scl")
                    nc.vector.tensor_tensor(
                        out=scl[:], in0=t1[:], in1=n2a[:], op=OP.mult
                    )
                    nc.vector.tensor_scalar(
                        out=scl[:], in0=scl[:], scalar1=1.0, scalar2=None, op0=OP.add
                    )
                    scl_bc = bass.AP(
                        scl.tensor, scl.offset, [scl.ap[0], scl.ap[1], [0, D]]
                    )
                    hth = pa.tile([128, SUP, D], f16, tag="hth")
                    # split the per-tile scale across Scalar (idle in phase A)
                    # and Vector to balance the phase-A critical path
                    for t in range(SUP):
                        if t % 2 == 0:
                            nc.scalar.activation(
                                hth[:, t, :], xin[:, t, :], AF.Copy,
                                scale=scl[:, t : t + 1],
                            )
                        else:
                            nc.vector.tensor_scalar(
                                out=hth[:, t, :], in0=xin[:, t, :],
                                scalar1=scl[:, t : t + 1], scalar2=None,
                                op0=OP.mult,
                            )
                    if s < NP1:
                        nc.sync.dma_start(
                            out=htab[
                                s * SUP * 128 : (s + 1) * SUP * 128, :
                            ].rearrange("(t p) d -> p t d", p=128),
                            in_=hth[:],
                        )
                    else:
                        j0 = (s - NP1) * SUP
                        nc.sync.dma_start(
                            out=htab_pj[:, j0 : j0 + SUP, :], in_=hth[:]
                        )
                    g0 = s * SUP
                    if g0 < NW:
                        nt = min(SUP, NW - g0)
                        scl_bc2 = bass.AP(
                            scl.tensor, scl.offset,
                            [scl.ap[0], [scl.ap[1][0], nt], [0, D]],
                        )
                        nc.vector.tensor_tensor(
                            out=h_loc[:, g0 : g0 + nt, :],
                            in0=xin[:, 0:nt, :], in1=scl_bc2, op=OP.mult,
                        )

            tc.strict_bb_all_engine_barrier()

            # ---------------- phase B/C: edges + per-window epilogue -------
            htab_lo = htab[0:HALF, :]
            htab_hi = htab[HALF:NROT, :]
            _regs = {}

            def nreg(v):
                if v not in _regs:
                    _regs[v] = nc.gpsimd.to_reg(v)
                return _regs[v]

            with (
                tc.tile_pool(name="pg", bufs=3) as pg,
                tc.tile_pool(name="phB", bufs=3) as pb,
                tc.tile_pool(name="chain", bufs=2) as pc,
                tc.tile_pool(name="scr", bufs=2) as scr,
                tc.tile_pool(name="psum", bufs=2, space="PSUM") as psp,
            ):
                kumax = max(ku)
                groups = [
                    list(range(g0, min(g0 + SW, NW)))
                    for g0 in range(0, NW, SW)
                ]
                hbmax = max(sum(ku[w] for w in ws) for ws in groups)
                # zero the gather buffers once: slots beyond the per-core
                # dynamic count stay finite (never NaN) for the poly chain
                for _ in range(3):
                    hbz = pg.tile([128, hbmax, D], f16, tag="hb")
                    nc.vector.memset(hbz[:], 0.0)
                icol = 0
                for ws in groups:
                    KA = sum(ka[w] for w in ws)
                    KB = sum(kb[w] for w in ws)
                    NT = KA + KB
                    nA, nB = KA * 128, KB * 128
                    ncol = (nA + nB) // 16
                    idx_t = pg.tile([128, (hbmax * 128) // 16], i16, tag="idxh")
                    nc.sync.dma_start(
                        out=idx_t[:, 0:ncol], in_=idxh_d[:, icol : icol + ncol]
                    )
                    icol += ncol
                    hbg = pg.tile([128, hbmax, D], f16, tag="hb")
                    rgA, rgB = nreg(nA), nreg(nB)
                    nc.gpsimd.dma_gather(
                        out_ap=hbg[:, 0:KA, :], in_ap=htab_lo,
                        idxs_ap=idx_t[:, 0 : nA // 16],
                        num_idxs=nA, num_idxs_reg=rgA,
                        elem_size=D, single_packet=False,
                    )
                    nc.gpsimd.dma_gather(
                        out_ap=hbg[:, KA:NT, :], in_ap=htab_hi,
                        idxs_ap=idx_t[:, nA // 16 : ncol],
                        num_idxs=nB, num_idxs_reg=rgB,
                        elem_size=D, single_packet=False,
                    )
                    aoff = 0
                    boff = KA
                    for w in ws:
                        kA, kB = ka[w], kb[w]
                        kU = kA + kB
                        wo = int(woff[w])
                        ps = psp.tile([128, 130], f32, tag="ps")
                        w_t = pb.tile([128, kumax, D], f16, tag="w_t")
                        nc.sync.dma_start(
                            out=w_t[:, 0:kU, :], in_=w_d[:, wo : wo + kU, :]
                        )
                        r_t = pb.tile([128, kumax, D], f16, tag="r_t")
                        nc.sync.dma_start(
                            out=r_t[:, 0:kU, :], in_=r_d[:, wo : wo + kU, :]
                        )
                        # msg = h*w + r (fp16, contiguous -> 2x DVE mode)
                        msg = pb.tile([128, kumax, D], f16, tag="msg")
                        nc.vector.tensor_tensor(
                            out=msg[:, 0:kA, :], in0=hbg[:, aoff : aoff + kA, :],
                            in1=w_t[:, 0:kA, :], op=OP.mult,
                        )
                        nc.vector.tensor_tensor(
                            out=msg[:, kA:kU, :], in0=hbg[:, boff : boff + kB, :],
                            in1=w_t[:, kA:kU, :], op=OP.mult,
                        )
                        aoff += kA
                        boff += kB
                        nc.vector.tensor_tensor(
                            out=msg[:, 0:kU, :], in0=msg[:, 0:kU, :],
                            in1=r_t[:, 0:kU, :], op=OP.add,
                        )
                    # n2 = |msg|^2 per edge
                    sqt = scr.tile([128, tpw, D], f16, tag="sqt")
                    nc.vector.tensor_tensor(
                        out=sqt[:, 0:kU, :], in0=msg[:, 0:kU, :],
                        in1=msg[:, 0:kU, :], op=OP.mult,
                    )
                    n2t = pc.tile([128, tpw], f32, tag="n2", name="n2")
                    n2 = n2t[:, 0:kU]
                    nc.vector.tensor_reduce(
                        out=n2, in_=sqt[:, 0:kU, :],
                        axis=mybir.AxisListType.X, op=OP.add,
                    )

                    def PCT(tag):
                        return pc.tile([128, tpw], f32, tag=tag, name=tag)

                    # polynomial chain in n2 (no activation tables)
                    Ax_t = PCT("Ax")
                    Ax = Ax_t[:, 0:kU]
                    nc.vector.tensor_scalar(
                        out=Ax, in0=n2, scalar1=P1, scalar2=1.0,
                        op0=OP.mult, op1=OP.add,
                    )
                    A2x = PCT("A2")[:, 0:kU]
                    nc.vector.tensor_tensor(out=A2x, in0=Ax, in1=Ax, op=OP.mult)
                    th2 = PCT("th2")[:, 0:kU]
                    nc.vector.scalar_tensor_tensor(
                        out=th2, in0=A2x, scalar=C, in1=n2,
                        op0=OP.mult, op1=OP.mult,
                    )
                    dn = PCT("dn")[:, 0:kU]
                    nc.vector.tensor_scalar(
                        out=dn, in0=th2, scalar1=-1.0, scalar2=1.0,
                        op0=OP.mult, op1=OP.add,
                    )
                    rd = PCT("rd")[:, 0:kU]
                    nc.vector.reciprocal(rd, dn)
                    sxi_t = PCT("sxi")
                    nc.vector.scalar_tensor_tensor(
                        out=sxi_t[:, 0:kU], in0=Ax, scalar=2.0, in1=rd,
                        op0=OP.mult, op1=OP.mult,
                    )
                    x0r = PCT("x0r")[:, 0:kU]
                    nc.vector.scalar_tensor_tensor(
                        out=x0r, in0=th2, scalar=1.0, in1=rd,
                        op0=OP.add, op1=OP.mult,
                    )
                    # rhs rows [1, x0, sxi*msg] (fp16)
                    rhs = pb.tile([128, tpw, 130], f16, tag="rhs")
                    nc.vector.memset(rhs[:, 0:kU, 0], 1.0)
                    nc.scalar.activation(
                        rhs[:, 0:kU, 1], x0r, AF.Copy, scale=1.0 / SC
                    )
                    for t in range(kU):
                        if t % 2 == 0:
                            nc.vector.tensor_scalar(
                                out=rhs[:, t, 2:130], in0=msg[:, t, :],
                                scalar1=sxi_t[:, t : t + 1], scalar2=None,
                                op0=OP.mult,
                            )
                        else:
                            nc.scalar.activation(
                                rhs[:, t, 2:130], msg[:, t, :], AF.Copy,
                                scale=sxi_t[:, t : t + 1],
                            )
                    # one-hot dst selection + segment-sum matmuls (fp16)
                    selc = scr.tile([128, tpw, 128], f16, tag="selc")
                    iota_bc = bass.AP(
                        iota_f.tensor, iota_f.offset,
                        [iota_f.ap[0], [0, kU], iota_f.ap[1]],
                    )
                    drel_sl = drel_sb[:, tpw * w : tpw * w + kU]
                    drel_bc = bass.AP(
                        drel_sl.tensor, drel_sl.offset,
                        [drel_sl.ap[0], drel_sl.ap[1], [0, 128]],
                    )
                    nc.vector.tensor_tensor(
                        out=selc[:, 0:kU, :], in0=iota_bc, in1=drel_bc,
                        op=OP.is_equal,
                    )
                    for t in range(kU):
                        nc.tensor.matmul(
                            ps[:], selc[:, t, :], rhs[:, t, :],
                            start=(t == 0), stop=(t == kU - 1),
                        )
                    # ---------------- phase C (per window) -----------------
                    nc.scalar.copy(S_all[:, w, :], ps[:])
                    sq2 = scr.tile([128, 129], f32, tag="sq2")
                    nc.scalar.activation(
                        sq2[:], S_all[:, w, 1:130], AF.Square,
                        accum_out=s2raw[:, w : w + 1],
                    )
                    tp = psp.tile([128, 128], f32, tag="tp")
                    nc.tensor.transpose(tp[:], h_loc[:, w, :], ident[:])
                    hT = scr.tile([128, 128], f32, tag="hT")
                    nc.vector.tensor_copy(out=hT[:], in_=tp[:])
                    lp = psp.tile([128, 128], f32, tag="lp")
                    nc.tensor.matmul(lp[:], hT[:], LW[:], start=True, stop=True)
                    ep = psp.tile([128, 128], f32, tag="ep")
                    nc.tensor.matmul(ep[:], hT[:], EV[:], start=True, stop=True)
                    mk = scr.tile([128, 1], i8, tag="mk")
                    nc.vector.tensor_scalar(
                        out=mk[:], in0=S_all[:, w, 0:1], scalar1=0.0,
                        scalar2=None, op0=OP.is_gt,
                    )
                    nc.scalar.copy(hn[:, w, :], ep[:])
                    nc.vector.copy_predicated(
                        out=hn[:, w, :], mask=mk[:].to_broadcast([128, 128]),
                        data=lp[:],
                    )

                # ---------------- phase D: per-node epilogue ---------------
                def B(tag):
                    return pc.tile([128, NW], f32, tag=tag, name=tag)[:]

                def TTb(dst, a, b, op):
                    nc.vector.tensor_tensor(out=dst, in0=a, in1=b, op=op)

                deg = S_all[:, :, 0]
                S0 = S_all[:, :, 1]
                q = B("Dq")
                TTb(q, norm_sb[:], deg, OP.mult)
                qq = B("Dqq")
                nc.vector.tensor_scalar(
                    out=qq, in0=q, scalar1=1e-6, scalar2=None, op0=OP.add
                )
                rq = B("Drq")
                nc.vector.reciprocal(rq, qq)
                fac = B("Dfac")
                TTb(fac, norm_sb[:], rq, OP.mult)
                mu0 = B("Dmu0")
                TTb(mu0, S0, fac, OP.mult)
                f2 = B("Df2")
                TTb(f2, fac, fac, OP.mult)
                s2 = B("Ds2")
                TTb(s2, s2raw[:], f2, OP.mult)
                m0s = B("Dm0s")
                TTb(m0s, mu0, mu0, OP.mult)
                mm = B("Dmm")
                nc.vector.tensor_scalar(
                    out=mm, in0=m0s, scalar1=-2.0, scalar2=None, op0=OP.mult
                )
                mink = B("Dmink")
                TTb(mink, s2, mm, OP.add)
                ab = B("Dab")
                nc.scalar.activation(ab, mink, AF.Abs)
                am = B("Dam")
                nc.vector.tensor_scalar(
                    out=am, in0=ab, scalar1=EPS, scalar2=None, op0=OP.max
                )
                sqm = B("Dsqm")
                nc.scalar.activation(sqm, am, AF.Sqrt)
                rr = B("Drr")
                nc.vector.reciprocal(rr, sqm)
                cf = B("Dcf")
                nc.vector.tensor_scalar(
                    out=cf, in0=rr, scalar1=1.0 / SC, scalar2=None, op0=OP.mult
                )
                c0 = B("Dc0")
                TTb(c0, mu0, cf, OP.mult)
                pd = B("Dpd")
                nc.vector.tensor_scalar(
                    out=pd, in0=c0, scalar1=SC, scalar2=1.0, op0=OP.mult, op1=OP.add
                )
                pdc = B("Dpdc")
                nc.vector.tensor_scalar(
                    out=pdc, in0=pd, scalar1=EPS, scalar2=None, op0=OP.max
                )
                rpd = B("Drpd")
                nc.vector.reciprocal(rpd, pdc)
                s_y = B("Dsy")
                TTb(s_y, cf, rpd, OP.mult)
                sp2 = B("Dsp2")
                TTb(sp2, s2, m0s, OP.subtract)
                y2 = B("Dy2")
                TTb(y2, s_y, s_y, OP.mult)
                ny2 = B("Dny2")
                TTb(ny2, y2, sp2, OP.mult)
                nyr = B("Dnyr")
                nc.scalar.activation(nyr, ny2, AF.Sqrt)
                ny = B("Dny")
                nc.vector.tensor_scalar(
                    out=ny, in0=nyr, scalar1=EPS, scalar2=None, op0=OP.max
                )
                v = B("Dv")
                nc.vector.tensor_scalar(
                    out=v, in0=ny, scalar1=SC, scalar2=1.0 - EPS,
                    op0=OP.mult, op1=OP.min,
                )
                la = B("Dla")
                nc.scalar.activation(la, v, AF.Ln, bias=1.0, scale=1.0)
                lb = B("Dlb")
                nc.scalar.activation(lb, v, AF.Ln, bias=1.0, scale=-1.0)
                df = B("Ddf")
                TTb(df, la, lb, OP.subtract)
                rny = B("Drny")
                nc.vector.reciprocal(rny, ny)
                k0 = B("Dk0")
                TTb(k0, df, rny, OP.mult)
                k1 = B("Dk1")
                TTb(k1, k0, s_y, OP.mult)
                k2 = B("Dk2")
                TTb(k2, k1, fac, OP.mult)
                hfac = pp.tile([128, NW], f32)
                nc.vector.tensor_scalar(
                    out=hfac[:], in0=k2, scalar1=0.5 / SC, scalar2=None, op0=OP.mult
                )
                for w in range(NW):
                    tmp = scr.tile([128, 128], f32, tag="d1")
                    hf_sl = hfac[:, w : w + 1]
                    hf_bc = bass.AP(
                        hf_sl.tensor, hf_sl.offset, [hf_sl.ap[0], [0, 128]]
                    )
                    nc.vector.tensor_tensor(
                        out=tmp[:], in0=S_all[:, w, 2:130], in1=hf_bc, op=OP.mult
                    )
                    nc.gpsimd.tensor_scalar(
                        out=tmp[:], in0=tmp[:], scalar1=10.0, scalar2=-10.0,
                        op0=OP.min, op1=OP.max,
                    )
                    nc.vector.tensor_tensor(
                        out=hn[:, w, :], in0=tmp[:], in1=hn[:, w, :], op=OP.add
                    )
                    nc.gpsimd.tensor_scalar(
                        out=hn[:, w, :], in0=hn[:, w, :], scalar1=10.0,
                        scalar2=-10.0, op0=OP.min, op1=OP.max,
                    )
                    sqd = scr.tile([128, 128], f32, tag="sqd")
                    nc.scalar.activation(
                        sqd[:], hn[:, w, :], AF.Square,
                        accum_out=ne2[:, w : w + 1],
                    )
                nnf = B("Dnnf")
                nc.scalar.activation(nnf, ne2[:], AF.Sqrt)
                nnc = B("Dnnc")
                nc.vector.tensor_scalar(
                    out=nnc, in0=nnf, scalar1=EPS, scalar2=None, op0=OP.max
                )
                thf = B("Dthf")
                nc.scalar.activation(thf, nnc, AF.Tanh, scale=SC)
                rnf = B("Drnf")
                nc.vector.reciprocal(rnf, nnc)
                sf0 = B("Dsf0")
                TTb(sf0, thf, rnf, OP.mult)
                sf = B("Dsf")
                nc.vector.tensor_scalar(
                    out=sf, in0=sf0, scalar1=1.0 / SC, scalar2=None, op0=OP.mult
                )
                for w in range(NW):
                    sf_sl = sf[:, w : w + 1]
                    sf_bc = bass.AP(
                        sf_sl.tensor, sf_sl.offset, [sf_sl.ap[0], [0, 128]]
                    )
                    nc.vector.tensor_tensor(
                        out=hn[:, w, :], in0=hn[:, w, :], in1=sf_bc, op=OP.mult
                    )
                nc.sync.dma_start(
                    out=out_d[:].rearrange("(w p) d -> p w d", p=128), in_=hn[:]
                )
    return nc


def get_program(tpw, ka, kb):
    global _PROGRAM
    key = (tpw, tuple(ka), tuple(kb))
    if _PROGRAM is None or _PROGRAM[1] != key:
        nc = _build_program(tpw, ka, kb)
        nc.compile()
        _PROGRAM = (nc, key)
    return _PROGRAM[0]


# ------------------------------------------------------------ host wrapper
def _preprocess(h_hyper, weight, loop_weight, evolve_loop_weight, rel_emb,
                norm, src, dst, etype):
    w_ext = np.concatenate(
        [weight.reshape(R, D).astype(np.float16), np.zeros((1, D), np.float16)]
    )
    r_ext = np.concatenate(
        [rel_emb.reshape(R, D).astype(np.float16), np.zeros((1, D), np.float16)]
    )
    h_pad = np.zeros((NROT, D), np.float32)
    h_pad[:N] = h_hyper
    src = src.astype(np.int64)
    dst = dst.astype(np.int64)
    core = dst // NPC
    local = dst - core * NPC
    win = local // 128
    rel = (local % 128).astype(np.float32)

    # pass 1: per-core per-window edge lists split by src half -> tile layout
    per_core = []
    for c in range(NCORES):
        m = core == c
        src_c, et_c, w_c, rel_c = src[m], etype[m], win[m], rel[m]
        rot = (src_c - c * NPC) % NROT
        half = (rot >= HALF).astype(np.int64)
        order = np.lexsort((half, w_c))
        per_core.append((rot[order], et_c[order], w_c[order], rel_c[order],
                         half[order]))

    ka = np.zeros(NW, np.int64)
    kb = np.zeros(NW, np.int64)
    for c in range(NCORES):
        rot_c, _, w_c, _, half_c = per_core[c]
        for w in range(NW):
            sel = w_c == w
            eA = int(np.sum(sel & (half_c == 0)))
            eB = int(np.sum(sel & (half_c == 1)))
            ka[w] = max(ka[w], (eA + 127) // 128)
            kb[w] = max(kb[w], (eB + 127) // 128)
    tpw = int((ka + kb).max())
    ku = ka + kb
    woff = np.concatenate([[0], np.cumsum(ku)]).astype(int)
    ntiles = int(woff[-1])
    eslot = ntiles * 128
    idxc = eslot // 16
    groups = [list(range(g0, min(g0 + SW, NW))) for g0 in range(0, NW, SW)]

    in_maps = []
    for c in range(NCORES):
        rot_c, et_c, w_c, rel_c, half_c = per_core[c]
        etyp = np.full(eslot, R, np.int16)
        drelf = np.full(eslot, -1.0, np.float32)
        cnt = np.zeros((1, 2 * NW), np.int32)
        # slot arrays: tight window-major [A(w) slots..., B(w) slots...]
        idx_by_w = {}
        for w in range(NW):
            sel = w_c == w
            rot_w, et_w, rel_w, half_w = (
                rot_c[sel], et_c[sel], rel_c[sel], half_c[sel],
            )
            base = int(woff[w]) * 128
            off = 0
            for h, k in ((0, ka[w]), (1, kb[w])):
                mm = half_w == h
                idx = rot_w[mm] - h * HALF
                cnt[0, 2 * w + h] = max(16, (len(idx) + 15) // 16 * 16)
                n = int(k) * 128
                ii = np.zeros(n, np.int16)
                ii[: len(idx)] = idx.astype(np.int16)
                idx_by_w[(w, h)] = ii
                lo = base + off
                etyp[lo : lo + len(idx)] = et_w[mm].astype(np.int16)
                drelf[lo : lo + len(idx)] = rel_w[mm]
                off += n
        # gather idx stream: per super-group, all A halves then all B halves
        idx_cols = []
        for ws in groups:
            for h in (0, 1):
                allidx = np.concatenate([idx_by_w[(w, h)] for w in ws])
                # 16-wrap: idx i -> (partition i%16, col i//16)
                idx_cols.append(allidx.reshape(-1, 16).T)
        big = np.concatenate(idx_cols, axis=1)  # [16, idxc]
        assert big.shape[1] == idxc
        idx_full = np.tile(big, (8, 1))

        w_e = np.ascontiguousarray(
            w_ext[etyp].reshape(ntiles, 128, D).transpose(1, 0, 2)
        )
        r_e = np.ascontiguousarray(
            r_ext[etyp].reshape(ntiles, 128, D).transpose(1, 0, 2)
        )

        n_real = min(NPC, N - c * NPC)
        norm_c = np.ones((NPC, 1), np.float32)
        norm_c[:n_real] = norm[c * NPC : c * NPC + n_real].astype(np.float32)

        hr = np.roll(h_pad, -c * NPC, axis=0).astype(np.float16)
        h_roll_t = np.concatenate([
            hr[:7168].reshape(56, 128, D).transpose(1, 0, 2),
            hr[7168:].reshape(128, NROT // 128 - 56, D),
        ], axis=1)

        in_maps.append({
            "h_roll": np.ascontiguousarray(h_roll_t),
            "w_e": w_e,
            "r_e": r_e,
            "lw": loop_weight.astype(np.float32),
            "ev": evolve_loop_weight.astype(np.float32),
            "norm_c": norm_c,
            "idx_h": idx_full,
            "cnt": cnt,
            "drel": drelf.reshape(ntiles, 128).T.copy(),
        })
    return in_maps, tpw, list(ka), list(kb)


def run(inputs, trace=False, **kw):
    from concourse.bass_utils import run_bass_kernel_spmd

    in_maps, tpw, ka, kb = _preprocess(**inputs)
    nc = get_program(tpw, ka, kb)
    res = run_bass_kernel_spmd(nc, in_maps, list(range(NCORES)), trace=trace, **kw)
    parts = []
    for c in range(NCORES):
        n_real = min(NPC, N - c * NPC)
        parts.append(res.results[c]["out"][:n_real])
    out = np.concatenate(parts, axis=0)
    return out, res


def kernel(**inputs) -> np.ndarray:
    out, _ = run(inputs)
    return out
